# revision 1
# baseline (speedup 1.0000x reference)
"""Trainium2 Bass kernel for causal multi-head attention (prefill).

Problem: x[2,2048,768], 12 heads x 64 dim, causal softmax(QK^T/8)V + out-proj.

Sharding (8 cores, no collectives): core c handles batch c//4 and head group
c%4 (3 heads).  Each core computes, for its batch b and heads hs:
    qT,kT = (Wq_hs @ x_b^T), (Wk_hs @ x_b^T)        [192, 2048] (transposed)
    v     = x_b @ Wv_hs^T                            [2048, 192+ones]
    expT  = exp(scoresT/8) masked causally           [kv, sq] per head
    ctxT_h = v_aug^T @ expT  (extra row = softmax denom via ones column)
    outT_partial = Wo[:,cols_hs] @ (ctxT/den)        [768, 2048]
Host sums the 4 partial outputs per batch and transposes back.

All matmuls run as float32r (full-rate fp32 on the PE at N>=256); every
tensor feeding a matmul is float32r end-to-end (walrus requires producers
to round to f32r).  Softmax skips the max-subtraction: scores/8 ~ N(0,1),
so exp stays in fp32 range.  Causal masking: below-diagonal blocks are
computed at partial width starting at the diagonal, the 128x128 diagonal
triangle is masked by multiplying with a host-provided 0/1 mask, and
above-diagonal regions are simply never computed nor read.
"""

import numpy as np

import concourse.bass as bass
import concourse.tile as tile
from concourse import bacc, mybir
from concourse.bass_utils import run_bass_kernel_spmd

F32 = mybir.dt.float32
F32R = mybir.dt.float32r

B, S, D = 2, 2048, 768
H, DH = 12, 64
HPC = 3                 # heads per core
GH = HPC * DH           # 192 head dims per core
NCORES = 8
KT = D // 128           # 6 contraction tiles for projections
NSQ = S // 512          # 4 sq blocks of 512
NKV = S // 128          # 16 kv tiles of 128
WJ = 1024               # exp/ctx window width
NJ = S // WJ            # 2 windows


def build():
    nc = bacc.Bacc("TRN2", target_bir_lowering=False, debug=False)

    xT = nc.dram_tensor("xT", [D, S], F32R, kind="ExternalInput")
    wq = nc.dram_tensor("wq", [D, GH], F32R, kind="ExternalInput")
    wk = nc.dram_tensor("wk", [D, GH], F32R, kind="ExternalInput")
    wv = nc.dram_tensor("wv", [D, 256], F32R, kind="ExternalInput")  # 192 + 64 pad
    wo = nc.dram_tensor("wo", [GH, D], F32R, kind="ExternalInput")
    tri = nc.dram_tensor("tri", [128, 128], F32R, kind="ExternalInput")
    onesd = nc.dram_tensor("onesd", [1, 64], F32R, kind="ExternalInput")
    outT = nc.dram_tensor("outT", [D, S], F32, kind="ExternalOutput")

    with tile.TileContext(nc) as tc, \
         nc.allow_low_precision(reason="fp32r tiles feeding fp32r matmuls"):
        with tc.tile_pool(name="sb", bufs=1) as sb, \
             tc.tile_pool(name="sbe", bufs=3) as sbe, \
             tc.tile_pool(name="sbo", bufs=2) as sbo, \
             tc.tile_pool(name="ps", bufs=2, space="PSUM") as ps, \
             tc.tile_pool(name="psc", bufs=1, space="PSUM") as psc:

            # ---- phase 0: load weights + x ----
            xsb = sb.tile([128, KT, S], F32R, tag="xsb")
            x_r = xT[:, :].rearrange("(k p) n -> p k n", p=128)
            for k in range(KT):
                nc.sync.dma_start(xsb[:, k, :], x_r[:, k, :])

            wq_sb = sb.tile([128, KT, GH], F32R, tag="wq")
            wk_sb = sb.tile([128, KT, GH], F32R, tag="wk")
            wv_sb = sb.tile([128, KT, 256], F32R, tag="wv")
            nc.sync.dma_start(wq_sb, wq[:, :].rearrange("(k p) m -> p k m", p=128))
            nc.sync.dma_start(wk_sb, wk[:, :].rearrange("(k p) m -> p k m", p=128))
            nc.sync.dma_start(wv_sb, wv[:, :].rearrange("(k p) m -> p k m", p=128))
            wo01_sb = sb.tile([128, D], F32R, tag="wo01")
            wo2_sb = sb.tile([64, D], F32R, tag="wo2")
            nc.sync.dma_start(wo01_sb, wo[0:128, :])
            nc.sync.dma_start(wo2_sb, wo[128:GH, :])
            tri_sb = sb.tile([128, 128], F32R, tag="tri")
            nc.sync.dma_start(tri_sb, tri[:, :])
            ones_sb = sb.tile([1, 64], F32R, tag="ones")
            nc.sync.dma_start(ones_sb, onesd[:, :])

            # ---- phase 1: projections ----
            # qT/kT: [192, S] as [128, 2, S] tiles (Mt0 = heads 0/1, Mt1 = head 2)
            qt_sb = sb.tile([128, 2, S], F32R, tag="qt")
            kt_sb = sb.tile([128, 2, S], F32R, tag="kt")
            for dst, wsb in ((qt_sb, wq_sb), (kt_sb, wk_sb)):
                for mt in range(2):          # 128 rows, then 64 rows
                    mp = 128 if mt == 0 else 64
                    for nt in range(NSQ):
                        pp = ps.tile([128, 512], F32, tag="sc", name="pp")
                        for k in range(KT):
                            nc.tensor.matmul(
                                pp[:mp, :],
                                wsb[:, k, mt * 128:mt * 128 + mp],
                                xsb[:, k, nt * 512:(nt + 1) * 512],
                                start=(k == 0), stop=(k == KT - 1))
                        nc.vector.tensor_copy(
                            dst[:mp, mt, nt * 512:(nt + 1) * 512], pp[:mp, :])

            # v_aug: [128, NKV, 195]; per kv tile: head h v at cols 65h..65h+63,
            # ones at col 65h+64 (written via ACT copy: tri*0 + 1)
            vaug = sb.tile([128, NKV, 195], F32R, tag="vaug")
            for h in range(HPC):
                nc.scalar.activation(
                    vaug[:, :, 65 * h + 64:65 * h + 65],
                    tri_sb[:, h * NKV:(h + 1) * NKV].rearrange(
                        "p (t c) -> p t c", c=1),
                    mybir.ActivationFunctionType.Copy, bias=1.0, scale=0.0)
            for i in range(NKV):
                pp = ps.tile([128, 256], F32, tag="sc", name="pp")
                for k in range(KT):
                    nc.tensor.matmul(
                        pp,
                        xsb[:, k, i * 128:(i + 1) * 128],
                        wv_sb[:, k, :],
                        start=(k == 0), stop=(k == KT - 1))
                nc.vector.tensor_copy(
                    vaug[:, i, :].rearrange("p (h c) -> p h c", c=65)[:, :, 0:64],
                    pp[:, 0:192].rearrange("p (h c) -> p h c", c=64))

            # ---- phase 2: attention ----
            # ctxT: heads 0/1 packed in one [128, S] tile (h1 via partition-
            # shifting sbuf-to-sbuf DMA), head 2 in its own [64, S] tile.
            ctxT01 = sb.tile([128, S], F32R, tag="ctxT01")
            ctxT2 = sb.tile([64, S], F32R, tag="ctxT2")
            for J in range(NJ):
                for h in range(HPC):
                    if h < 2:
                        def kslc(i, h=h):
                            return kt_sb[64 * h:64 * h + 64, 0, i * 128:(i + 1) * 128]

                        def qslc(c0, c1, h=h):
                            return qt_sb[64 * h:64 * h + 64, 0, c0:c1]
                    else:
                        def kslc(i):
                            return kt_sb[0:64, 1, i * 128:(i + 1) * 128]

                        def qslc(c0, c1):
                            return qt_sb[0:64, 1, c0:c1]

                    ctx_ps = psc.tile([65, WJ], F32, tag="ctx", name="ctx_ps")
                    imax = 8 * J + 7
                    for i in range(imax + 1):
                        d = 128 * i - WJ * J       # window col of diagonal start
                        col0 = max(0, d)
                        nb0 = max(0, d // 512)
                        spsum = ps.tile([128, WJ], F32, tag="sc", name="spsum")
                        for nb in range(nb0, 2):
                            s0 = max(nb * 512, col0)
                            nc.tensor.matmul(
                                spsum[:, s0:(nb + 1) * 512],
                                kslc(i),
                                qslc(WJ * J + s0, WJ * J + (nb + 1) * 512),
                                start=True, stop=True)
                        esb = sbe.tile([128, WJ], F32R, tag="exp", name="esb")
                        nc.scalar.activation(
                            esb[:, col0:WJ], spsum[:, col0:WJ],
                            mybir.ActivationFunctionType.Exp, scale=0.125)
                        if d >= 0:
                            nc.vector.tensor_mul(
                                esb[:, d:d + 128], esb[:, d:d + 128], tri_sb)
                        for nb in range(nb0, 2):
                            s0 = max(nb * 512, col0)
                            nc.tensor.matmul(
                                ctx_ps[:, s0:(nb + 1) * 512],
                                vaug[:, i, 65 * h:65 * h + 65],
                                esb[:, s0:(nb + 1) * 512],
                                start=(i == 0), stop=(i == 8 * J + 4 * nb + 3))
                    # normalize rows 0:64 by row 64 (softmax denominator)
                    h1tmp = (sbo.tile([64, WJ], F32R, tag="h1tmp", name="h1tmp")
                             if h == 1 else None)
                    for nb in range(2):
                        c0 = WJ * J + nb * 512
                        inv = sbo.tile([1, 512], F32R, tag="inv", name="inv")
                        nc.vector.reciprocal(
                            inv, ctx_ps[64:65, nb * 512:(nb + 1) * 512])
                        bps = ps.tile([64, 512], F32, tag="sc", name="bps")
                        nc.tensor.matmul(bps, ones_sb, inv, start=True, stop=True)
                        bsb = sbo.tile([64, 512], F32, tag="bsb", name="bsb")
                        nc.vector.tensor_copy(bsb, bps)
                        if h == 0:
                            dst = ctxT01[0:64, c0:c0 + 512]
                        elif h == 1:
                            dst = h1tmp[:, nb * 512:(nb + 1) * 512]
                        else:
                            dst = ctxT2[:, c0:c0 + 512]
                        nc.vector.tensor_mul(
                            dst, ctx_ps[0:64, nb * 512:(nb + 1) * 512], bsb)
                    if h == 1:
                        # partition-shift h1's ctxT into rows 64:128
                        nc.sync.dma_start(
                            ctxT01[64:128, WJ * J:WJ * (J + 1)], h1tmp)

                # ---- phase 3: out-projection for this window ----
                for j in (2 * J, 2 * J + 1):
                    for mt in range(6):
                        ops = ps.tile([128, 512], F32, tag="sc", name="ops")
                        nc.tensor.matmul(
                            ops, wo01_sb[:, mt * 128:(mt + 1) * 128],
                            ctxT01[:, j * 512:(j + 1) * 512],
                            start=True, stop=False)
                        nc.tensor.matmul(
                            ops, wo2_sb[:, mt * 128:(mt + 1) * 128],
                            ctxT2[:, j * 512:(j + 1) * 512],
                            start=False, stop=True)
                        osb = sbo.tile([128, 512], F32, tag="osb", name="osb")
                        nc.vector.tensor_copy(osb, ops)
                        nc.sync.dma_start(
                            outT[mt * 128:(mt + 1) * 128, j * 512:(j + 1) * 512],
                            osb)

    nc.compile()
    return nc


def shard_inputs(x, Wq, Wk, Wv, Wo):
    x = np.asarray(x, np.float32)
    tri = np.triu(np.ones((128, 128), np.float32))
    ones = np.ones((1, 64), np.float32)
    in_maps = []
    for c in range(NCORES):
        b, g = c // 4, c % 4
        rs = slice(GH * g, GH * g + GH)
        wv_t = np.concatenate(
            [np.ascontiguousarray(np.asarray(Wv, np.float32)[rs].T),
             np.zeros((D, 64), np.float32)], axis=1)
        in_maps.append({
            "xT": np.ascontiguousarray(x[b].T),
            "wq": np.ascontiguousarray(np.asarray(Wq, np.float32)[rs].T),
            "wk": np.ascontiguousarray(np.asarray(Wk, np.float32)[rs].T),
            "wv": wv_t,
            "wo": np.ascontiguousarray(np.asarray(Wo, np.float32)[:, rs].T),
            "tri": tri,
            "onesd": ones,
        })
    return in_maps


def assemble(results, bo):
    out = np.zeros((B, S, D), np.float32)
    for c in range(NCORES):
        out[c // 4] += results[c]["outT"].T
    return out + np.asarray(bo, np.float32)[None, None, :]


_NC = None


def kernel(x, Wq, Wk, Wv, Wo, bo, **run_kwargs):
    global _NC
    if _NC is None:
        _NC = build()
    in_maps = shard_inputs(x, Wq, Wk, Wv, Wo)
    res = run_bass_kernel_spmd(_NC, in_maps, core_ids=list(range(NCORES)),
                               **run_kwargs)
    out = assemble(res.results, bo)
    kernel.last_results = res
    return out



# revision 7
# speedup vs baseline: 1.2631x; 1.2631x over previous
"""Trainium2 Bass kernel for causal multi-head attention (prefill), fp16.

Problem: x[2,2048,768], 12 heads x 64 dim, causal softmax(QK^T/8)V + out-proj.

Sharding (8 cores, no collectives): core c handles batch c//4 and head group
c%4 (3 heads).  Each core computes, for its batch b and heads hs:
    qkT  = (Wqk_hs @ x_b^T)                [384, 2048] packed M-tiles
    v    = x_b @ Wv_hs^T                   [2048, 192] interleaved with ones
    expT = exp(scoresT/8) masked causally  [kv, q] per head, fp16
    ctxT_h = v_aug^T @ expT  (extra ones row -> softmax denom)
    outT_partial = Wo[:,cols_hs] @ (ctxT/den)   [768, 2048] fp16
Host sums the 4 partial outputs per batch and transposes back.

All matmul operands are fp16 (psum accumulates fp32): fp16 moving operands
stream at 1 row/cycle (fp32 c=128 needs 2 cycles/row due to the 256B/cycle
SBUF feed limit), LDWEIGHTS is half cost, DMA halves, and the shorter
kernel avoids the mid-kernel clock throttle the fp32 version hit.
Softmax skips max-subtraction: scores/8 ~ N(0,1), exp fits fp16 range.
Denominator reciprocal uses the fast approx DVE op (~18 bits, plenty for
the 2e-2 gate) broadcast across partitions via a c=1 ones matmul.
"""

import numpy as np

import concourse.bass as bass
import concourse.tile as tile
from concourse import bacc, mybir
from concourse.bass_utils import run_bass_kernel_spmd

F32 = mybir.dt.float32
F32R = mybir.dt.float32r
F16 = mybir.dt.float16

B, S, D = 2, 2048, 768
H, DH = 12, 64
HPC = 3                 # heads per core
GH = HPC * DH           # 192 head dims per core
NCORES = 8
KT = D // 128           # 6 contraction tiles for projections
W = 512                 # q window width
NW = S // W             # 4 windows
NKV = S // 128          # 16 kv tiles of 128


def build():
    nc = bacc.Bacc("TRN2", target_bir_lowering=False, debug=False)

    xT = nc.dram_tensor("xT", [D, S], F16, kind="ExternalInput")
    wqk = nc.dram_tensor("wqk", [D, 384], F16, kind="ExternalInput")
    wv = nc.dram_tensor("wv", [D, GH], F16, kind="ExternalInput")
    wo = nc.dram_tensor("wo", [GH, D], F16, kind="ExternalInput")
    tri = nc.dram_tensor("tri", [128, 128], F16, kind="ExternalInput")
    onesd = nc.dram_tensor("onesd", [1, 64], F16, kind="ExternalInput")
    outT = nc.dram_tensor("outT", [D, S], F16, kind="ExternalOutput")

    with tile.TileContext(nc) as tc, \
         nc.allow_low_precision(reason="fp16 tiles feeding fp16 matmuls"):
        with tc.tile_pool(name="sb", bufs=1) as sb, \
             tc.tile_pool(name="sbe", bufs=3) as sbe, \
             tc.tile_pool(name="sbo", bufs=3) as sbo, \
             tc.tile_pool(name="ps", bufs=3, space="PSUM") as ps, \
             tc.tile_pool(name="psc", bufs=2, space="PSUM") as psc, \
             tc.tile_pool(name="psb", bufs=2, space="PSUM") as psb:

            # ---- phase 0: load weights + x ----
            xsb = sb.tile([128, KT, S], F16, tag="xsb")
            x_r = xT[:, :].rearrange("(k p) n -> p k n", p=128)
            for k in range(KT):
                nc.sync.dma_start(xsb[:, k, :], x_r[:, k, :])

            wqk_sb = sb.tile([128, KT, 384], F16, tag="wqk")
            wv_sb = sb.tile([128, KT, GH], F16, tag="wv")
            nc.sync.dma_start(wqk_sb, wqk[:, :].rearrange("(k p) m -> p k m", p=128))
            nc.sync.dma_start(wv_sb, wv[:, :].rearrange("(k p) m -> p k m", p=128))
            wo01_sb = sb.tile([128, D], F16, tag="wo01")
            wo2_sb = sb.tile([64, D], F16, tag="wo2")
            nc.sync.dma_start(wo01_sb, wo[0:128, :])
            nc.sync.dma_start(wo2_sb, wo[128:GH, :])
            tri_sb = sb.tile([128, 128], F16, tag="tri")
            nc.sync.dma_start(tri_sb, tri[:, :])
            ones_sb = sb.tile([1, 64], F16, tag="ones")
            nc.sync.dma_start(ones_sb, onesd[:, :])

            # vaug ones columns (written before v-proj copies; disjoint cols)
            vaug = sb.tile([128, NKV, 195], F16, tag="vaug")
            for h in range(HPC):
                nc.scalar.activation(
                    vaug[:, :, 65 * h + 64:65 * h + 65],
                    tri_sb[:, h * NKV:(h + 1) * NKV].rearrange(
                        "p (t c) -> p t c", c=1),
                    mybir.ActivationFunctionType.Copy, bias=1.0, scale=0.0)

            # ---- phase 1: projections ----
            # qkT: [384, S] as [128, 3, S]; col blocks of wqk:
            #   mt0 = q heads 0,1 | mt1 = k heads 0,1 | mt2 = [q2 | k2]
            qkt = sb.tile([128, 3, S], F16, tag="qkt")
            for mt in range(3):
                for nt in range(NW):
                    pp = ps.tile([128, W], F32, tag="sc", name="pp")
                    for k in range(KT):
                        nc.tensor.matmul(
                            pp,
                            wqk_sb[:, k, mt * 128:(mt + 1) * 128],
                            xsb[:, k, nt * W:(nt + 1) * W],
                            start=(k == 0), stop=(k == KT - 1))
                    nc.vector.tensor_copy(
                        qkt[:, mt, nt * W:(nt + 1) * W], pp)
            # k head 2 shifted to partitions 0:64 (scores need q2/k2 on the
            # same base partition; they share mt2, so copy k2 down)
            kk2 = sb.tile([64, S], F16, tag="kk2")
            nc.sync.dma_start(kk2, qkt[64:128, 2, :])

            # v_aug: [128, NKV, 195]; per kv tile: head h v at cols 65h..65h+63,
            # ones at col 65h+64
            for i in range(NKV):
                pp = ps.tile([128, GH], F32, tag="sc", name="pp")
                for k in range(KT):
                    nc.tensor.matmul(
                        pp,
                        xsb[:, k, i * 128:(i + 1) * 128],
                        wv_sb[:, k, :],
                        start=(k == 0), stop=(k == KT - 1))
                nc.vector.tensor_copy(
                    vaug[:, i, :].rearrange("p (h c) -> p h c", c=65)[:, :, 0:64],
                    pp.rearrange("p (h c) -> p h c", c=64))

            def kslc(h, i):
                if h == 0:
                    return qkt[0:64, 1, i * 128:(i + 1) * 128]
                if h == 1:
                    return qkt[64:128, 1, i * 128:(i + 1) * 128]
                return kk2[:, i * 128:(i + 1) * 128]

            def qslc(h, c0, c1):
                if h == 0:
                    return qkt[0:64, 0, c0:c1]
                if h == 1:
                    return qkt[64:128, 0, c0:c1]
                return qkt[0:64, 2, c0:c1]

            # ---- phase 2: attention ----
            ctxT01 = sb.tile([128, S], F16, tag="ctxT01")
            ctxT2 = sb.tile([64, S], F16, tag="ctxT2")
            for J in range(NW):
                w0 = J * W
                for h in range(HPC):
                    ctx_ps = psc.tile([65, W], F32, tag="ctx", name="ctx_ps")
                    imax = 4 * J + 3
                    for i in range(imax + 1):
                        d = 128 * i - w0      # window col of diagonal start
                        col0 = max(0, d)
                        spsum = ps.tile([128, W], F32, tag="sc", name="spsum")
                        nc.tensor.matmul(
                            spsum[:, col0:W],
                            kslc(h, i),
                            qslc(h, w0 + col0, w0 + W),
                            start=True, stop=True)
                        esb = sbe.tile([128, W], F16, tag="exp", name="esb")
                        nc.scalar.activation(
                            esb[:, col0:W], spsum[:, col0:W],
                            mybir.ActivationFunctionType.Exp, scale=0.125)
                        if d >= 0:
                            nc.vector.tensor_mul(
                                esb[:, d:d + 128], esb[:, d:d + 128], tri_sb)
                        nc.tensor.matmul(
                            ctx_ps[:, col0:W],
                            vaug[:, i, 65 * h:65 * h + 65],
                            esb[:, col0:W],
                            start=(i == 0), stop=(i == imax))
                    # normalize rows 0:64 by row 64 (softmax denominator)
                    inv = sbo.tile([1, W], F32, tag="inv", name="inv")
                    nc.vector.reciprocal(inv, ctx_ps[64:65, :])
                    inv16 = sbo.tile([1, W], F16, tag="inv16", name="inv16")
                    nc.scalar.activation(inv16, inv,
                                         mybir.ActivationFunctionType.Copy)
                    bps = psb.tile([64, W], F32, tag="bps", name="bps")
                    nc.tensor.matmul(bps, ones_sb, inv16,
                                     start=True, stop=True)
                    bsb = sbo.tile([64, W], F16, tag="bsb", name="bsb")
                    nc.vector.tensor_copy(bsb, bps)
                    if h == 0:
                        dst = ctxT01[0:64, w0:w0 + W]
                    elif h == 1:
                        dst = sbo.tile([64, W], F16, tag="h1t", name="h1tmp")
                    else:
                        dst = ctxT2[:, w0:w0 + W]
                    nc.vector.tensor_mul(dst, ctx_ps[0:64, :], bsb)
                    if h == 1:
                        # partition-shift h1's ctxT into rows 64:128
                        nc.sync.dma_start(ctxT01[64:128, w0:w0 + W], dst)

                # ---- phase 3: out-projection for this window ----
                for mt in range(6):
                    ops = ps.tile([128, W], F32, tag="sc", name="ops")
                    nc.tensor.matmul(
                        ops, wo01_sb[:, mt * 128:(mt + 1) * 128],
                        ctxT01[:, w0:w0 + W],
                        start=True, stop=False)
                    nc.tensor.matmul(
                        ops, wo2_sb[:, mt * 128:(mt + 1) * 128],
                        ctxT2[:, w0:w0 + W],
                        start=False, stop=True)
                    osb = sbo.tile([128, W], F16, tag="osb", name="osb")
                    nc.vector.tensor_copy(osb, ops)
                    nc.sync.dma_start(
                        outT[mt * 128:(mt + 1) * 128, w0:w0 + W], osb)

    nc.compile()
    return nc


def shard_inputs(x, Wq, Wk, Wv, Wo):
    x = np.asarray(x, np.float32)
    Wq = np.asarray(Wq, np.float32)
    Wk = np.asarray(Wk, np.float32)
    Wv = np.asarray(Wv, np.float32)
    Wo = np.asarray(Wo, np.float32)
    tri = np.triu(np.ones((128, 128), np.float16))
    ones = np.ones((1, 64), np.float16)
    in_maps = []
    for c in range(NCORES):
        b, g = c // 4, c % 4
        r0 = GH * g
        wqk = np.concatenate([
            Wq[r0:r0 + 128].T,          # q heads 0,1
            Wk[r0:r0 + 128].T,          # k heads 0,1
            Wq[r0 + 128:r0 + 192].T,    # q head 2
            Wk[r0 + 128:r0 + 192].T,    # k head 2
        ], axis=1).astype(np.float16)
        in_maps.append({
            "xT": np.ascontiguousarray(x[b].T).astype(np.float16),
            "wqk": np.ascontiguousarray(wqk),
            "wv": np.ascontiguousarray(Wv[r0:r0 + GH].T).astype(np.float16),
            "wo": np.ascontiguousarray(Wo[:, r0:r0 + GH].T).astype(np.float16),
            "tri": tri,
            "onesd": ones,
        })
    return in_maps


def assemble(results, bo):
    out = np.zeros((B, S, D), np.float32)
    for c in range(NCORES):
        out[c // 4] += results[c]["outT"].astype(np.float32).T
    return out + np.asarray(bo, np.float32)[None, None, :]


_NC = None


def kernel(x, Wq, Wk, Wv, Wo, bo, **run_kwargs):
    global _NC
    if _NC is None:
        _NC = build()
    in_maps = shard_inputs(x, Wq, Wk, Wv, Wo)
    res = run_bass_kernel_spmd(_NC, in_maps, core_ids=list(range(NCORES)),
                               **run_kwargs)
    out = assemble(res.results, bo)
    kernel.last_results = res
    return out


# revision 11
# speedup vs baseline: 1.4104x; 1.1167x over previous
"""Trainium2 Bass kernel for causal multi-head attention (prefill), fp16.

Problem: x[2,2048,768], 12 heads x 64 dim, causal softmax(QK^T/8)V + out-proj.

Sharding (8 cores, no collectives): core c handles batch c//4 and head group
c%4 (3 heads).  Each core computes, for its batch b and heads hs:
    qkT  = (Wqk_hs @ x_b^T)                [384, 2048] packed M-tiles
    v    = x_b @ Wv_hs^T                   [2048, 192] interleaved with ones
    expT = exp(scoresT/8) masked causally  [kv, q] per head, fp16
    ctxT_h = v_aug^T @ expT  (extra ones row -> softmax denom)
    outT_partial = Wo[:,cols_hs] @ (ctxT/den)   [768, 2048] fp16
Host sums the 4 partial outputs per batch and transposes back.

All matmul operands are fp16 (psum accumulates fp32): fp16 moving operands
stream at 1 row/cycle (fp32 c=128 needs 2 cycles/row due to the 256B/cycle
SBUF feed limit), LDWEIGHTS is half cost, DMA halves, and the shorter
kernel avoids the mid-kernel clock throttle the fp32 version hit.
Softmax skips max-subtraction: scores/8 ~ N(0,1), exp fits fp16 range.
Denominator reciprocal uses the fast approx DVE op (~18 bits, plenty for
the 2e-2 gate) broadcast across partitions via a c=1 ones matmul.
"""

import numpy as np

import concourse.bass as bass
import concourse.tile as tile
from concourse import bacc, mybir
from concourse.bass_utils import run_bass_kernel_spmd

F32 = mybir.dt.float32
F32R = mybir.dt.float32r
F16 = mybir.dt.float16

B, S, D = 2, 2048, 768
H, DH = 12, 64
HPC = 3                 # heads per core
GH = HPC * DH           # 192 head dims per core
NCORES = 8
KT = D // 128           # 6 contraction tiles for projections
W = 512                 # q window width
NW = S // W             # 4 windows
NKV = S // 128          # 16 kv tiles of 128


def build():
    nc = bacc.Bacc("TRN2", target_bir_lowering=False, debug=False)

    xT = nc.dram_tensor("xT", [D, S], F16, kind="ExternalInput")
    wqk = nc.dram_tensor("wqk", [D, 384], F16, kind="ExternalInput")
    wv = nc.dram_tensor("wv", [D, GH], F16, kind="ExternalInput")
    wo = nc.dram_tensor("wo", [GH, D], F16, kind="ExternalInput")
    tri = nc.dram_tensor("tri", [128, 128], F16, kind="ExternalInput")
    onesd = nc.dram_tensor("onesd", [1, 64], F16, kind="ExternalInput")
    outT = nc.dram_tensor("outT", [D, S], F16, kind="ExternalOutput")

    with tile.TileContext(nc) as tc, \
         nc.allow_low_precision(reason="fp16 tiles feeding fp16 matmuls"):
        with tc.tile_pool(name="sb", bufs=1) as sb, \
             tc.tile_pool(name="sbe", bufs=3) as sbe, \
             tc.tile_pool(name="sbo", bufs=3) as sbo, \
             tc.tile_pool(name="ps", bufs=2, space="PSUM") as ps, \
             tc.tile_pool(name="pso", bufs=2, space="PSUM") as pso, \
             tc.tile_pool(name="psc", bufs=2, space="PSUM") as psc, \
             tc.tile_pool(name="psb", bufs=2, space="PSUM") as psb:

            # ---- phase 0: load weights + x ----
            xsb = sb.tile([128, KT, S], F16, tag="xsb")
            x_r = xT[:, :].rearrange("(k p) n -> p k n", p=128)
            for k in range(KT):
                nc.sync.dma_start(xsb[:, k, :], x_r[:, k, :])

            # PE warmup during the DMA phase: the HAM clock gate starts at
            # 1.2 GHz and only releases to 2.4 GHz after ~3.4us of sustained
            # PE activity.  Burn dummy matmuls on a zeroed tile so the real
            # projections start at full clock instead of paying the cold
            # penalty.
            warm = sb.tile([128, 512], F16, tag="warm")
            nc.vector.memset(warm, 0.0)
            for _ in range(20):
                wps = pso.tile([128, 512], F32, tag="ops", name="wps")
                nc.tensor.matmul(wps, warm[:, 0:128], warm,
                                 start=True, stop=True)

            wqk_sb = sb.tile([128, KT, 384], F16, tag="wqk")
            wv_sb = sb.tile([128, KT, GH], F16, tag="wv")
            nc.sync.dma_start(wqk_sb, wqk[:, :].rearrange("(k p) m -> p k m", p=128))
            nc.sync.dma_start(wv_sb, wv[:, :].rearrange("(k p) m -> p k m", p=128))
            wo01_sb = sb.tile([128, D], F16, tag="wo01")
            wo2_sb = sb.tile([64, D], F16, tag="wo2")
            nc.sync.dma_start(wo01_sb, wo[0:128, :])
            nc.sync.dma_start(wo2_sb, wo[128:GH, :])
            tri_sb = sb.tile([128, 128], F16, tag="tri")
            nc.sync.dma_start(tri_sb, tri[:, :])
            ones_sb = sb.tile([1, 64], F16, tag="ones")
            nc.sync.dma_start(ones_sb, onesd[:, :])

            # vaug ones columns (written before v-proj copies; disjoint cols)
            vaug = sb.tile([128, NKV, 195], F16, tag="vaug")
            for h in range(HPC):
                nc.scalar.activation(
                    vaug[:, :, 65 * h + 64:65 * h + 65],
                    tri_sb[:, h * NKV:(h + 1) * NKV].rearrange(
                        "p (t c) -> p t c", c=1),
                    mybir.ActivationFunctionType.Copy, bias=1.0, scale=0.0)

            # ---- phase 1: projections ----
            # qkT: [384, S] as [128, 3, S]; col blocks of wqk:
            #   mt0 = q heads 0,1 | mt1 = k heads 0,1 | mt2 = [q2 | k2]
            qkt = sb.tile([128, 3, S], F16, tag="qkt")
            for mt in range(3):
                for nt in range(NW):
                    pp = ps.tile([128, W], F32, tag="sc", name="pp")
                    for k in range(KT):
                        nc.tensor.matmul(
                            pp,
                            wqk_sb[:, k, mt * 128:(mt + 1) * 128],
                            xsb[:, k, nt * W:(nt + 1) * W],
                            start=(k == 0), stop=(k == KT - 1))
                    nc.vector.tensor_copy(
                        qkt[:, mt, nt * W:(nt + 1) * W], pp)
            # k head 2 shifted to partitions 0:64 (scores need q2/k2 on the
            # same base partition; they share mt2, so copy k2 down)
            kk2 = sb.tile([64, S], F16, tag="kk2")
            nc.sync.dma_start(kk2, qkt[64:128, 2, :])

            # v_aug: [128, NKV, 195]; per kv tile: head h v at cols 65h..65h+63,
            # ones at col 65h+64
            for i in range(NKV):
                pp = ps.tile([128, GH], F32, tag="sc", name="pp")
                for k in range(KT):
                    nc.tensor.matmul(
                        pp,
                        xsb[:, k, i * 128:(i + 1) * 128],
                        wv_sb[:, k, :],
                        start=(k == 0), stop=(k == KT - 1))
                nc.vector.tensor_copy(
                    vaug[:, i, :].rearrange("p (h c) -> p h c", c=65)[:, :, 0:64],
                    pp.rearrange("p (h c) -> p h c", c=64))

            def kslc(h, i):
                if h == 0:
                    return qkt[0:64, 1, i * 128:(i + 1) * 128]
                if h == 1:
                    return qkt[64:128, 1, i * 128:(i + 1) * 128]
                return kk2[:, i * 128:(i + 1) * 128]

            def qslc(h, c0, c1):
                if h == 0:
                    return qkt[0:64, 0, c0:c1]
                if h == 1:
                    return qkt[64:128, 0, c0:c1]
                return qkt[0:64, 2, c0:c1]

            # ---- phase 2: attention ----
            ctxT01 = sb.tile([128, S], F16, tag="ctxT01")
            ctxT2 = sb.tile([64, S], F16, tag="ctxT2")
            for J in range(NW):
                w0 = J * W
                for h in range(HPC):
                    ctx_ps = psc.tile([65, W], F32, tag="ctx", name="ctx_ps")
                    imax = 4 * J + 3
                    for i in range(imax + 1):
                        d = 128 * i - w0      # window col of diagonal start
                        col0 = max(0, d)
                        spsum = ps.tile([128, W], F32, tag="sc", name="spsum")
                        nc.tensor.matmul(
                            spsum[:, col0:W],
                            kslc(h, i),
                            qslc(h, w0 + col0, w0 + W),
                            start=True, stop=True)
                        esb = sbe.tile([128, W], F16, tag="exp", name="esb")
                        nc.scalar.activation(
                            esb[:, col0:W], spsum[:, col0:W],
                            mybir.ActivationFunctionType.Exp, scale=0.125)
                        if d >= 0:
                            nc.vector.tensor_mul(
                                esb[:, d:d + 128], esb[:, d:d + 128], tri_sb)
                        nc.tensor.matmul(
                            ctx_ps[:, col0:W],
                            vaug[:, i, 65 * h:65 * h + 65],
                            esb[:, col0:W],
                            start=(i == 0), stop=(i == imax))
                    # normalize rows 0:64 by row 64 (softmax denominator)
                    den = sbo.tile([1, W], F32, tag="den", name="den")
                    nc.scalar.activation(den, ctx_ps[64:65, :],
                                         mybir.ActivationFunctionType.Copy)
                    inv = sbo.tile([1, W], F32, tag="inv", name="inv")
                    nc.vector.reciprocal_approx_fast(inv, den)
                    inv16 = sbo.tile([1, W], F16, tag="inv16", name="inv16")
                    nc.scalar.activation(inv16, inv,
                                         mybir.ActivationFunctionType.Copy)
                    bps = psb.tile([64, W], F32, tag="bps", name="bps")
                    nc.tensor.matmul(bps, ones_sb, inv16,
                                     start=True, stop=True)
                    bsb = sbo.tile([64, W], F16, tag="bsb", name="bsb")
                    nc.vector.tensor_copy(bsb, bps)
                    if h == 0:
                        dst = ctxT01[0:64, w0:w0 + W]
                    elif h == 1:
                        dst = sbo.tile([64, W], F16, tag="h1t", name="h1tmp")
                    else:
                        dst = ctxT2[:, w0:w0 + W]
                    nc.vector.tensor_mul(dst, ctx_ps[0:64, :], bsb)
                    if h == 1:
                        # partition-shift h1's ctxT into rows 64:128
                        nc.sync.dma_start(ctxT01[64:128, w0:w0 + W], dst)

                # ---- phase 3: out-projection for this window ----
                for mt in range(6):
                    ops = pso.tile([128, W], F32, tag="ops", name="ops")
                    nc.tensor.matmul(
                        ops, wo01_sb[:, mt * 128:(mt + 1) * 128],
                        ctxT01[:, w0:w0 + W],
                        start=True, stop=False)
                    nc.tensor.matmul(
                        ops, wo2_sb[:, mt * 128:(mt + 1) * 128],
                        ctxT2[:, w0:w0 + W],
                        start=False, stop=True)
                    osb = sbo.tile([128, W], F16, tag="osb", name="osb")
                    nc.vector.tensor_copy(osb, ops)
                    nc.sync.dma_start(
                        outT[mt * 128:(mt + 1) * 128, w0:w0 + W], osb)

    nc.compile()
    return nc


def shard_inputs(x, Wq, Wk, Wv, Wo):
    x = np.asarray(x, np.float32)
    Wq = np.asarray(Wq, np.float32)
    Wk = np.asarray(Wk, np.float32)
    Wv = np.asarray(Wv, np.float32)
    Wo = np.asarray(Wo, np.float32)
    tri = np.triu(np.ones((128, 128), np.float16))
    ones = np.ones((1, 64), np.float16)
    in_maps = []
    for c in range(NCORES):
        b, g = c // 4, c % 4
        r0 = GH * g
        wqk = np.concatenate([
            Wq[r0:r0 + 128].T,          # q heads 0,1
            Wk[r0:r0 + 128].T,          # k heads 0,1
            Wq[r0 + 128:r0 + 192].T,    # q head 2
            Wk[r0 + 128:r0 + 192].T,    # k head 2
        ], axis=1).astype(np.float16)
        in_maps.append({
            "xT": np.ascontiguousarray(x[b].T).astype(np.float16),
            "wqk": np.ascontiguousarray(wqk),
            "wv": np.ascontiguousarray(Wv[r0:r0 + GH].T).astype(np.float16),
            "wo": np.ascontiguousarray(Wo[:, r0:r0 + GH].T).astype(np.float16),
            "tri": tri,
            "onesd": ones,
        })
    return in_maps


def assemble(results, bo):
    out = np.zeros((B, S, D), np.float32)
    for c in range(NCORES):
        out[c // 4] += results[c]["outT"].astype(np.float32).T
    return out + np.asarray(bo, np.float32)[None, None, :]


_NC = None


def kernel(x, Wq, Wk, Wv, Wo, bo, **run_kwargs):
    global _NC
    if _NC is None:
        _NC = build()
    in_maps = shard_inputs(x, Wq, Wk, Wv, Wo)
    res = run_bass_kernel_spmd(_NC, in_maps, core_ids=list(range(NCORES)),
                               **run_kwargs)
    out = assemble(res.results, bo)
    kernel.last_results = res
    return out
